# revision 22
# baseline (speedup 1.0000x reference)
"""Causal multi-head self-attention on 8 TRN2 NeuronCores.

Problem: x (2, 2048, 1024) f32; w_q/w_k/w_v/w_o (1024, 1024) f32;
out = CausalMHA(x) (torch nn.Linear convention, 16 heads, d_k = 64).

Sharding (tensor-parallel over heads x data-parallel over batch):
core c -> batch bc = c//4, head group hg = c%4 (4 heads = 256 features).
Each core computes Q/K/V projections for its slice, causal attention for
its 4 heads, and a partial output projection against its w_o column
slice. The host sums the 4 partials per batch (the tensor-parallel
"all-reduce" of the w_o matmul, done host-side during unshard).

Device kernel (per core, one NEFF, all matmuls bf16 w/ f32 PSUM accum):
- scores computed K-major (ST layout [k, q]) so softmax'd P lands
  pre-transposed for the P^T V matmul; row-sums ride along as a ones
  column appended to V (stationary M=65)
- softmax without max subtraction (inputs bounded, exp args within +-6);
  exp on ScalarE reads straight from PSUM
- causal masking: band tiles use narrowed matmuls/exp plus a triangular
  affine_select zeroing on GPSIMD
- the two heads of a pair are issued to PE row groups (0,0)/(64,0) and
  run concurrently in the systolic array (K=64 each)
- projections / attention / output-projection are statically software-
  pipelined: proj and out-proj sub-groups (~0.5us each) are emitted as
  filler between attention k-tile groups so TensorE stays dense.  The
  pacing guarantees every queued filler unit is emitted within the
  chunk it was queued for (a unit emitted later than the instructions
  that read its output would silently read stale SBUF).
- normalization (batched per head-pair): sum rows -> partition 0 (DVE
  copies), one fast reciprocal over both halves, one GPSIMD partition
  broadcast, then two DVE multiplies reading the AV PSUM directly and
  writing the scaled bf16 YT.
  NOTE: the custom DVE/GPSIMD ops (reciprocal_approx_fast,
  partition_broadcast) require base_partition-0 APs on real HW;
  CoreSim does not model this.
- warm-up during the input-DMA phase: ACT exp table, the GPSIMD ucode
  library for PartitionBroadcast (first use otherwise stalls the whole
  GPSIMD queue ~7us mid-kernel on a LIBRARY_RELOAD), and K=128 dummy
  matmuls that keep the PE HAM clock-gate at 8/8 (K=1 matmuls do NOT
  register as PE activity) so the first real matmuls run at 2.4 GHz
- x is staged in DRAM chunk-major ([P, chunk, ka, TC]) so each chunk
  half loads as one contiguous 4KB-per-partition DMA; everything goes
  on the sync HWDGE ring in priority order (wq, x0, wk, wv, x1, wo,
  x2, x3) -- a parallel SWDGE ring would steal DMA-engine slots from
  the critical x-chunk-0 load
- output returned bf16 (cast to f32 host-side); out DMAs are split per
  512-column half across the sync and scalar HWDGE rings.
"""

import numpy as np
import ml_dtypes

import concourse.bass as bass
import concourse.tile as tile
from concourse import bacc, mybir
from concourse.bass import ts

P = 128
D = 1024          # d_model
T = 2048          # seq len
B = 2
NH = 4            # heads per core
DK = 64
F = NH * DK       # 256 local features
TC = 512          # token chunk (matmul N)
NCHUNK = T // TC  # 4
NTT = T // P      # 16 token tiles
KA = D // P       # 8 dmodel chunks
SCALE = 1.0 / np.sqrt(DK)

BF16 = mybir.dt.bfloat16
F32 = mybir.dt.float32
EXP = mybir.ActivationFunctionType.Exp


def build_nc():
    nc = bacc.Bacc(None, target_bir_lowering=False)
    with tile.TileContext(nc) as tc:
        with tc.tile_pool(name="dram", bufs=1, space="DRAM") as dram:
            xT = dram.tile((P, NCHUNK, KA, TC), BF16, kind="ExternalInput", name="xT", uniquify=False)
            wqT = dram.tile((P, 2, KA, P), BF16, kind="ExternalInput", name="wqT", uniquify=False)
            wkT = dram.tile((P, KA, F), BF16, kind="ExternalInput", name="wkT", uniquify=False)
            wvT = dram.tile((P, KA, F), BF16, kind="ExternalInput", name="wvT", uniquify=False)
            woT = dram.tile((P, F // P, D), BF16, kind="ExternalInput", name="woT", uniquify=False)
            out = dram.tile((P, NTT, D), BF16, kind="ExternalOutput", name="out", uniquify=False)

            with tc.tile_pool(name="big", bufs=1) as big:
                xT_sb = big.tile([P, NCHUNK, KA, TC], BF16)
                wqT_sb = big.tile([P, 2, KA, P], BF16)
                wkT_sb = big.tile([P, KA, F], BF16)
                wvT_sb = big.tile([P, KA, F], BF16)
                woT_sb = big.tile([P, F // P, D], BF16)
                QT_sb = big.tile([P, 2, T], BF16)   # head pair-major
                KT_sb = big.tile([P, 2, T], BF16)
                V_sb = big.tile([P, NTT, NH, 66], BF16)  # +ones col at 64
                YT_sb = big.tile([P, 2, T], BF16)

                nc.sync.dma_start(out=wqT_sb[:, 0], in_=wqT[:, 0])
                nc.sync.dma_start(out=xT_sb[:, 0, 0:4, :], in_=xT[:, 0, 0:4, :])
                nc.sync.dma_start(out=wqT_sb[:, 1], in_=wqT[:, 1])
                nc.sync.dma_start(out=xT_sb[:, 0, 4:8, :], in_=xT[:, 0, 4:8, :])
                nc.sync.dma_start(out=wkT_sb[:], in_=wkT[:])
                nc.sync.dma_start(out=wvT_sb[:], in_=wvT[:])
                nc.sync.dma_start(out=xT_sb[:, 1, 0:4, :], in_=xT[:, 1, 0:4, :])
                nc.sync.dma_start(out=xT_sb[:, 1, 4:8, :], in_=xT[:, 1, 4:8, :])
                nc.sync.dma_start(out=woT_sb[:], in_=woT[:])
                for n in range(2, NCHUNK):
                    nc.sync.dma_start(out=xT_sb[:, n, 0:4, :], in_=xT[:, n, 0:4, :])
                    nc.sync.dma_start(out=xT_sb[:, n, 4:8, :], in_=xT[:, n, 4:8, :])
                nc.gpsimd.memset(V_sb[:, :, :, 64:66], 1.0)

                with (tc.tile_pool(name="flex", bufs=2, space="PSUM") as flexp,
                      tc.tile_pool(name="st", bufs=2, space="PSUM") as stp,
                      tc.tile_pool(name="av", bufs=2, space="PSUM") as avp,
                      tc.tile_pool(name="pt", bufs=6) as ptp,
                      tc.tile_pool(name="sm", bufs=2) as smp,
                      tc.tile_pool(name="warm", bufs=1) as warmp,
                      tc.tile_pool(name="ob", bufs=3) as obp):

                    # ---- warm-up during the DMA phase ----
                    # ACT exp table
                    wt = warmp.tile([1, 8], F32)
                    nc.vector.memset(wt[:], 0.0)
                    nc.scalar.activation(wt[:], wt[:], EXP, scale=1.0)
                    # GPSIMD ucode libraries (PartitionBroadcast +
                    # affine_select): first use otherwise reloads ucode
                    # mid-kernel, stalling the GPSIMD queue ~7us.
                    wpb_in = warmp.tile([1, 8], F32)
                    wpb_out = warmp.tile([64, 8], F32)
                    nc.vector.memset(wpb_in[:], 1.0)
                    nc.gpsimd.partition_broadcast(wpb_out[:], wpb_in[:])
                    nc.gpsimd.affine_select(
                        out=wpb_out[0:64, 0:8], in_=wpb_out[0:64, 0:8],
                        compare_op=mybir.AluOpType.is_ge, fill=0.0,
                        base=0, pattern=[[1, 8]], channel_multiplier=-1)
                    # PE HAM warm-up: K=128 dummy matmuls (K=1 does not
                    # count as PE activity for the clock gate).
                    wls = warmp.tile([P, P], BF16)
                    wmr = warmp.tile([P, TC], BF16)
                    nc.vector.memset(wls[:], 0.0)
                    nc.vector.memset(wmr[:], 0.0)
                    # one accumulation group into a single PSUM tile so
                    # the warm-up does not hold both flex slots hostage
                    # (WAR) when the first real projection is ready
                    wps = flexp.tile([P, TC], F32, name="flex")
                    for i in range(8):
                        nc.tensor.matmul(wps[:], lhsT=wls[:], rhs=wmr[:],
                                         start=(i == 0), stop=(i == 7))

                    stash = {}

                    def qk_sub(n, pr, which, half):
                        a0 = 4 * half

                        def emit():
                            if half == 0:
                                ps = stash["qk"] = flexp.tile([P, TC], F32, name="flex")
                            else:
                                ps = stash["qk"]
                            for a in range(a0, a0 + 4):
                                lhsT = (wqT_sb[:, pr, a, :] if which == "q"
                                        else wkT_sb[:, a, ts(pr, P)])
                                nc.tensor.matmul(
                                    ps[:], lhsT=lhsT,
                                    rhs=xT_sb[:, n, a, :],
                                    start=(a == 0), stop=(a == KA - 1))
                            if half == 1:
                                dst = QT_sb if which == "q" else KT_sb
                                nc.vector.tensor_copy(dst[:, pr, ts(n, TC)], ps[:])
                        return emit

                    def v_sub(tt, half):
                        a0 = 4 * half

                        def emit():
                            if half == 0:
                                ps_v = stash["v"] = flexp.tile([P, TC], F32, name="flex")
                            else:
                                ps_v = stash["v"]
                            for a in range(a0, a0 + 4):
                                nc.tensor.matmul(
                                    ps_v[:, 0:F], lhsT=xT_sb[:, tt // 4, a, ts(tt % 4, P)],
                                    rhs=wvT_sb[:, a, :],
                                    start=(a == 0), stop=(a == KA - 1))
                            if half == 1:
                                nc.vector.tensor_copy(
                                    V_sb[:, tt, :, 0:64],
                                    ps_v[:, 0:F].rearrange("p (h d) -> p h d", h=NH))
                        return emit

                    def proj_groups(n):
                        gs = [qk_sub(n, pr, w, h)
                              for w in ("q", "k") for pr in range(2) for h in range(2)]
                        gs += [v_sub(tt, h) for tt in range(4 * n, 4 * n + 4)
                               for h in range(2)]
                        return gs

                    def proj(n):
                        for g in proj_groups(n):
                            g()

                    filler = []

                    def normalize2(av_a, av_b, pr, n, tailwarm=False):
                        # per-half chains (copy -> reciprocal -> broadcast
                        # -> multiply) interleaved on DVE/GPSIMD so the
                        # first half's AV PSUM frees early and the YT
                        # halves land as soon as possible.  Mid-chunk the
                        # AV PSUMs are evacuated (ScalarE/DVE) so the next
                        # head-pair's first AV matmuls are not gated on
                        # the full chain; at the tail nothing waits on
                        # the slots, so the multiplies read PSUM direct.
                        srow_a = smp.tile([1, TC], F32, name="srow_a")
                        srow_b = smp.tile([1, TC], F32, name="srow_b")
                        rec_a = smp.tile([1, TC], F32, name="rec_a")
                        rec_b = smp.tile([1, TC], F32, name="rec_b")
                        rb_a = smp.tile([1, TC], BF16, name="rb_a")
                        rb_b = smp.tile([1, TC], BF16, name="rb_b")
                        r64_a = smp.tile([64, TC], BF16, name="r64_a")
                        r64_b = smp.tile([64, TC], BF16, name="r64_b")
                        nc.vector.tensor_copy(srow_a[:], av_a[64:65, :])
                        nc.vector.tensor_copy(srow_b[:], av_b[64:65, :])
                        nc.vector.reciprocal_approx_fast(out=rec_a[:], in_=srow_a[:])
                        nc.vector.tensor_copy(rb_a[:], rec_a[:])
                        # bf16 broadcast: half the GPSIMD time, so the next
                        # head-pair's band masks queue behind less work
                        nc.gpsimd.partition_broadcast(r64_a[:], rb_a[:])
                        if not tailwarm:
                            sb_a = smp.tile([64, TC], F32, name="sb_a")
                            sb_b = smp.tile([64, TC], F32, name="sb_b")
                            nc.scalar.copy(sb_a[:], av_a[0:64, :])
                        nc.vector.reciprocal_approx_fast(out=rec_b[:], in_=srow_b[:])
                        nc.vector.tensor_copy(rb_b[:], rec_b[:])
                        if not tailwarm:
                            nc.vector.tensor_copy(sb_b[:], av_b[0:64, :])
                        nc.gpsimd.partition_broadcast(r64_b[:], rb_b[:])
                        if tailwarm:
                            # junk K=64 matmuls dependent on r64_a/r64_b:
                            # keep the PE HAM window alive through the
                            # serial chain before the final out-proj
                            jp = flexp.tile([P, TC], F32, name="flex")
                            nc.tensor.matmul(jp[:], lhsT=r64_a[0:64, 0:P],
                                             rhs=r64_a[0:64, 0:TC],
                                             start=True, stop=True)
                        src_a = av_a[0:64, :] if tailwarm else sb_a[:]
                        src_b = av_b[0:64, :] if tailwarm else sb_b[:]
                        nc.vector.tensor_mul(
                            YT_sb[0:64, pr, ts(n, TC)], src_a, r64_a[:])
                        nc.vector.tensor_mul(
                            YT_sb[64:128, pr, ts(n, TC)], src_b, r64_b[:])

                    def attention(n, tail_units=()):
                        # take ownership of everything queued: all of it
                        # MUST be emitted within this chunk (later chunks'
                        # instructions read these units' outputs).
                        due = list(filler)
                        filler.clear()
                        emitted = 0
                        last_kt = 4 * n + 3
                        ngroups = 2 * (4 * n + 4)
                        gi = 0
                        for hp in range(2):
                            av_a = avp.tile([65, TC], F32, name="av_ps")
                            av_b = avp.tile([65, TC], F32, name="av_ps")

                            pending = []  # deferred AV emission: 1-deep
                            # software pipeline so the next k-tile's score
                            # matmuls sit between exp(kt) and AV(kt) on
                            # the PE queue (no per-tile exp-wait bubble)

                            def flush_av():
                                for kt_, band_, s_, pt_ in pending:
                                    for j, av in ((0, av_a), (1, av_b)):
                                        if band_:
                                            nc.gpsimd.affine_select(
                                                out=pt_[:, j, s_:s_ + 128],
                                                in_=pt_[:, j, s_:s_ + 128],
                                                compare_op=mybir.AluOpType.is_ge,
                                                fill=0.0, base=0,
                                                pattern=[[1, 128]],
                                                channel_multiplier=-1)
                                        nc.tensor.matmul(
                                            av[:, s_:TC],
                                            lhsT=V_sb[:, kt_, 2 * hp + j, 0:65],
                                            rhs=pt_[:, j, s_:TC],
                                            start=(kt_ == 0), stop=(kt_ == last_kt))
                                pending.clear()

                            for kt in range(4 * n + 4):
                                jj = kt - 4 * n
                                band = (jj >= 0)
                                s = 128 * jj if band else 0
                                st_ps = stp.tile([P, 2, TC], F32, name="st_ps")
                                pt_sb = ptp.tile([P, 2, TC], BF16, name="pt_sb")
                                for j, r in ((0, 0), (1, 64)):
                                    nc.tensor.matmul(
                                        st_ps[:, j, s:TC],
                                        lhsT=KT_sb[r:r + 64, hp, ts(kt, P)],
                                        rhs=QT_sb[r:r + 64, hp, n * TC + s:(n + 1) * TC],
                                        start=True, stop=True)
                                nc.scalar.activation(
                                    pt_sb[:, :, s:TC], st_ps[:, :, s:TC],
                                    EXP, scale=float(SCALE))
                                # filler (and the previous tile's AV) land
                                # between this tile's scores and its AV:
                                # TensorE chews them while ScalarE
                                # exponentiates. Pace so all due units are
                                # emitted by the last slot.
                                gi += 1
                                target = (len(due) * gi) // ngroups
                                while emitted < target:
                                    due[emitted]()
                                    emitted += 1
                                flush_av()
                                pending.append((kt, band, s, pt_sb))
                            flush_av()
                            if hp == 1:
                                for g in tail_units:
                                    g()
                            normalize2(av_a, av_b, hp, n, tailwarm=(n == NCHUNK - 1))
                        while emitted < len(due):
                            due[emitted]()
                            emitted += 1

                    def op_sub(tt, half):
                        def emit():
                            _op_half(tt, half)
                        return emit

                    def _op_evac(tt, half, ps):
                        o_sb = stash.get(("o", tt))
                        if o_sb is None:
                            o_sb = stash[("o", tt)] = obp.tile([P, 2, TC], BF16, name="o_sb")
                        ring = nc.sync if half == 0 else nc.scalar
                        nc.vector.tensor_copy(o_sb[:, half, 0:256], ps[:, 0:256])
                        if half == 0:
                            # sync-ring DMA issue doesn't contend with the
                            # ScalarE copy below; the scalar-ring issue
                            # would, so half 1 defers both DMAs
                            ring.dma_start(out=out[:, tt, 0:256],
                                           in_=o_sb[:, half, 0:256])
                        nc.scalar.copy(o_sb[:, half, 256:TC], ps[:, 256:TC])
                        if half == 1:
                            ring.dma_start(out=out[:, tt, TC:TC + 256],
                                           in_=o_sb[:, half, 0:256])
                        ring.dma_start(out=out[:, tt, half * TC + 256:(half + 1) * TC],
                                       in_=o_sb[:, half, 256:TC])
                        if half == 1:
                            del stash[("o", tt)]

                    def outproj(n):
                        # final-chunk out-proj: the a=0 accumulation
                        # matmuls depend only on the first head-pair's YT
                        # (ready mid-chunk), so they execute during the
                        # tail normalize chain; only the a=1 matmuls and
                        # evacuations wait for the last YT half.  The
                        # first three tiles borrow PSUM from the st/flex
                        # pools (idle by now) to hold 6 open accumulators.
                        t0 = 4 * n
                        accs = []
                        for i, tt in enumerate((t0, t0 + 1)):
                            pst = stp.tile([P, 2, TC], F32, name="st_ps")
                            accs.extend((tt, h, pst[:, h, :]) for h in range(2))
                        for h in range(2):
                            pf = flexp.tile([P, TC], F32, name="flex")
                            accs.append((t0 + 2, h, pf[:]))
                        for tt, h, ps in accs:
                            nc.tensor.matmul(ps, lhsT=YT_sb[:, 0, ts(tt, P)],
                                             rhs=woT_sb[:, 0, ts(h, TC)],
                                             start=True, stop=False)
                        for tt, h, ps in accs:
                            nc.tensor.matmul(ps, lhsT=YT_sb[:, 1, ts(tt, P)],
                                             rhs=woT_sb[:, 1, ts(h, TC)],
                                             start=False, stop=True)
                            _op_evac(tt, h, ps)
                        _op_half(t0 + 3, 0, tail=True)
                        _op_half(t0 + 3, 1, tail=True)

                    def _op_half(tt, half, tail=False):
                        op_ps = flexp.tile([P, TC], F32, name="flex")
                        for a in range(2):
                            nc.tensor.matmul(
                                op_ps[:],
                                lhsT=YT_sb[:, a, ts(tt, P)],
                                rhs=woT_sb[:, a, ts(half, TC)],
                                start=(a == 0), stop=(a == 1))
                        o_sb = stash.get(("o", tt))
                        if o_sb is None:
                            o_sb = stash[("o", tt)] = obp.tile([P, 2, TC], BF16, name="o_sb")
                        if tail:
                            # split the PSUM evacuation across DVE and
                            # ScalarE so the flex slot frees in ~half the
                            # time (the final out-proj is evac-paced)
                            nc.vector.tensor_copy(o_sb[:, half, 0:256], op_ps[:, 0:256])
                            ring = nc.sync if half == 0 else nc.scalar
                            if tt >= 14 and half == 0:
                                # last tiles: DMA per quarter so the final
                                # write starts as early as possible (the
                                # scalar ring defers to avoid delaying the
                                # ScalarE quarter-copy behind a DGE issue)
                                ring.dma_start(
                                    out=out[:, tt, 0:256],
                                    in_=o_sb[:, half, 0:256])
                            nc.scalar.copy(o_sb[:, half, 256:TC], op_ps[:, 256:TC])
                            if tt >= 14:
                                if half == 1:
                                    ring.dma_start(
                                        out=out[:, tt, TC:TC + 256],
                                        in_=o_sb[:, half, 0:256])
                                ring.dma_start(
                                    out=out[:, tt, half * TC + 256:(half + 1) * TC],
                                    in_=o_sb[:, half, 256:TC])
                            else:
                                ring.dma_start(
                                    out=out[:, tt, half * TC:(half + 1) * TC],
                                    in_=o_sb[:, half, :])
                            if half == 1:
                                del stash[("o", tt)]
                        elif half == 0:
                            nc.vector.tensor_copy(o_sb[:, 0, :], op_ps[:])
                            nc.sync.dma_start(out=out[:, tt, 0:TC], in_=o_sb[:, 0, :])
                        else:
                            nc.scalar.copy(o_sb[:, 1, :], op_ps[:])
                            nc.scalar.dma_start(out=out[:, tt, TC:2 * TC], in_=o_sb[:, 1, :])
                            del stash[("o", tt)]

                    proj(0)
                    reserved = []
                    for n in range(NCHUNK):
                        if n + 1 < NCHUNK:
                            filler.extend(proj_groups(n + 1))
                        if n == NCHUNK - 1:
                            # hold back a few chunk-2 out-proj units to
                            # bridge the tail normalize chain (PE work
                            # that is ready the moment the last AV ends)
                            reserved = filler[-4:]
                            del filler[-4:]
                        attention(n, tail_units=reserved)
                        if n < NCHUNK - 1:
                            filler.extend(op_sub(tt, h)
                                          for tt in range(4 * n, 4 * n + 4)
                                          for h in range(2))
                        else:
                            outproj(n)
    nc.compile()
    return nc


# ---------------- host-side shard / gather + entry point ----------------

_NC_CACHE = []


def _part(a, p=P):
    """(p*chunks, rest...) -> (p, chunks, rest...) with partition inner."""
    k, rest = a.shape[0], a.shape[1:]
    return np.ascontiguousarray(
        a.reshape(k // p, p, *rest).transpose(1, 0, *range(2, a.ndim + 1)))


def _shard_inputs(x, w_q, w_k, w_v, w_o):
    bf = ml_dtypes.bfloat16
    in_maps = []
    xT_b = []
    for b in range(B):
        xp = _part(np.ascontiguousarray(np.asarray(x)[b].T).astype(bf))  # [P, KA, T]
        xp = np.ascontiguousarray(
            xp.reshape(P, KA, NCHUNK, TC).transpose(0, 2, 1, 3))  # [P, NCHUNK, KA, TC]
        xT_b.append(xp)
    w_q, w_k, w_v, w_o = (np.asarray(w) for w in (w_q, w_k, w_v, w_o))
    for c in range(8):
        bc, hg = c // 4, c % 4
        r0 = hg * F
        wqp = _part(np.ascontiguousarray(w_q[r0:r0 + F].T).astype(bf))  # [P, KA, F]
        wqp = np.ascontiguousarray(
            wqp.reshape(P, KA, 2, P).transpose(0, 2, 1, 3))  # [P, 2(pr), KA, 128]
        in_maps.append({
            "xT": xT_b[bc],
            "wqT": wqp,
            "wkT": _part(np.ascontiguousarray(w_k[r0:r0 + F].T).astype(bf)),
            "wvT": _part(np.ascontiguousarray(w_v[r0:r0 + F].T).astype(bf)),
            "woT": _part(np.ascontiguousarray(w_o[:, r0:r0 + F].T).astype(bf)),
        })
    return in_maps


def _gather(results):
    out = np.zeros((B, T, D), np.float32)
    for c in range(8):
        bc = c // 4
        part = np.asarray(results[c]["out"]).astype(np.float32).reshape(P, NTT, D)
        out[bc] += part.transpose(1, 0, 2).reshape(T, D)
    return out


def kernel(x, w_q, w_k, w_v, w_o):
    from concourse.bass_utils import run_bass_kernel_spmd
    if not _NC_CACHE:
        _NC_CACHE.append(build_nc())
    nc = _NC_CACHE[0]
    in_maps = _shard_inputs(x, w_q, w_k, w_v, w_o)
    res = run_bass_kernel_spmd(nc, in_maps, core_ids=list(range(8)))
    return _gather(res.results)
